# revision 5
# baseline (speedup 1.0000x reference)
"""Bass/Trainium2 distributed kernel for nn_BiDirectionalLoss.

Reference computation (see problem):
    feature1 = feat[:1024], feature2 = feat[1024:]
    dis = cdist(f1, f2)                                  # [B, B]
    half-1: row-wise masked max/argmax & min/argmin of dis over columns
    half-2: identical with roles swapped -> column-wise stats of the SAME dis
    cross  = mean(relu(furthest_pos - closest_neg + 0.5))
    intra  = mean(relu(0.1 - dis_intra[pos_pos, pos_neg]))
    loss   = cross1 + cross2 + 0.5 * (intra1 + intra2)

Distribution: core k owns row-block k of dis (half-1) and column-block k
(half-2, computed as a fresh [128, 1024] block of the transposed product so
all reductions are free-axis).  Each core computes its 128 rows of stats,
gathers the two intra rows per index via indirect DMA from HBM, produces a
weighted partial sum, and one AllGather + local reduce yields the scalar on
every core.
"""

import sys

if "/opt/trn_rl_repo" not in sys.path:
    sys.path.insert(0, "/opt/trn_rl_repo")

import numpy as np

P = 128          # partitions / block rows per core
B = 1024         # batch size
NCORES = 8
MARGIN = 0.5
NEG_OFFSET = 100000.0
INTRA_MARGIN = 0.1
EPS = 1e-12
LAMDA = 0.5

_CACHE = {}


def _build():
    import concourse.bacc as bacc
    import concourse.bass as bass
    import concourse.mybir as mybir
    import concourse.tile as tile

    f32 = mybir.dt.float32
    u32 = mybir.dt.uint32

    nc = bacc.Bacc("TRN2", target_bir_lowering=False, debug=False,
                   num_devices=NCORES)

    # Shared (identical on every core) inputs.
    f1T_d = nc.dram_tensor("f1T", [P, B], f32, kind="ExternalInput")
    f2T_d = nc.dram_tensor("f2T", [P, B], f32, kind="ExternalInput")
    feat1_d = nc.dram_tensor("feat1", [B, P], f32, kind="ExternalInput")
    feat2_d = nc.dram_tensor("feat2", [B, P], f32, kind="ExternalInput")
    lab1r_d = nc.dram_tensor("lab1r", [1, B], f32, kind="ExternalInput")
    lab2r_d = nc.dram_tensor("lab2r", [1, B], f32, kind="ExternalInput")
    # Per-core (sharded) inputs: this core's 128-column block of fT / labels.
    f1Tb_d = nc.dram_tensor("f1Tb", [P, P], f32, kind="ExternalInput")
    f2Tb_d = nc.dram_tensor("f2Tb", [P, P], f32, kind="ExternalInput")
    lab1b_d = nc.dram_tensor("lab1b", [P, 1], f32, kind="ExternalInput")
    lab2b_d = nc.dram_tensor("lab2b", [P, 1], f32, kind="ExternalInput")
    out_d = nc.dram_tensor("out", [1, 1], f32, kind="ExternalOutput")

    with tile.TileContext(nc) as tc:
        with (
            tc.tile_pool(name="sb", bufs=1) as sb,
            tc.tile_pool(name="ps", space="PSUM", bufs=1) as ps,
            tc.tile_pool(name="dr", space="DRAM", bufs=1) as dr,
        ):
            # ---- loads ----
            f1T = sb.tile([P, B], f32, name="f1T_sb")
            f2T = sb.tile([P, B], f32, name="f2T_sb")
            f1Tb = sb.tile([P, P], f32, name="f1Tb_sb")
            f2Tb = sb.tile([P, P], f32, name="f2Tb_sb")
            lab1b = sb.tile([P, 1], f32, name="lab1b_sb")
            lab2b = sb.tile([P, 1], f32, name="lab2b_sb")
            lab1B = sb.tile([P, B], f32, name="lab1B_sb")
            lab2B = sb.tile([P, B], f32, name="lab2B_sb")
            nc.sync.dma_start(f1T[:], f1T_d[:])
            nc.sync.dma_start(f2T[:], f2T_d[:])
            nc.sync.dma_start(f1Tb[:], f1Tb_d[:])
            nc.sync.dma_start(f2Tb[:], f2Tb_d[:])
            nc.sync.dma_start(lab1b[:], lab1b_d[:])
            nc.sync.dma_start(lab2b[:], lab2b_d[:])
            # label rows broadcast down all 128 partitions (stride-0 read)
            nc.sync.dma_start(lab1B[:], lab1r_d[0:1, :].to_broadcast([P, B]))
            nc.sync.dma_start(lab2B[:], lab2r_d[0:1, :].to_broadcast([P, B]))

            ones = sb.tile([P, 1], f32, name="ones_sb")
            nc.vector.memset(ones[:], 1.0)
            # activation() bias operands must be SBUF APs for non-Copy funcs
            eps_t = sb.tile([P, 1], f32, name="eps_sb")
            nc.vector.memset(eps_t[:], EPS)
            margin_t = sb.tile([P, 1], f32, name="margin_sb")
            nc.vector.memset(margin_t[:], MARGIN)
            intram_t = sb.tile([P, 1], f32, name="intram_sb")
            nc.vector.memset(intram_t[:], INTRA_MARGIN)
            ones_row = sb.tile([1, B], f32, name="ones_row_sb")
            nc.vector.memset(ones_row[:], 1.0)

            # ---- squared-norm rows: nrow_s[0, j] = ||f_s[j]||^2 ----
            nrow = []
            for s_i, fT in ((0, f1T), (1, f2T)):
                sq = sb.tile([P, B], f32, name=f"sq{s_i}_sb")
                nc.scalar.square(sq[:], fT[:])
                nr = sb.tile([1, B], f32, name=f"nrow{s_i}_sb")
                for c in range(2):
                    pnr = ps.tile([1, 512], f32, name=f"pnr{s_i}{c}",
                                  tag="pnorm", bufs=2)
                    nc.tensor.matmul(pnr[:], lhsT=ones[:],
                                     rhs=sq[:, c * 512:(c + 1) * 512],
                                     start=True, stop=True)
                    nc.scalar.copy(nr[0:1, c * 512:(c + 1) * 512], pnr[:])
                nrow.append(nr)

            # block-norm rows: nblk_s[0, i] = ||f_s[block_i]||^2
            nblk = []
            for s_i, fTb in ((0, f1Tb), (1, f2Tb)):
                sqb = sb.tile([P, P], f32, name=f"sqb{s_i}_sb")
                nc.scalar.square(sqb[:], fTb[:])
                pnb = ps.tile([1, P], f32, name=f"pnb{s_i}", tag="pnorm", bufs=2)
                nc.tensor.matmul(pnb[:], lhsT=ones[:], rhs=sqb[:],
                                 start=True, stop=True)
                nb = sb.tile([1, P], f32, name=f"nblk{s_i}_sb")
                nc.scalar.copy(nb[:], pnb[:])
                nblk.append(nb)

            # ---- per-half pipeline ----
            stats = sb.tile([P, 4], f32, name="stats_sb")  # [ct0, ct1, it0, it1]
            for h in range(2):
                if h == 0:   # rows of f1 vs all f2
                    aTb, labA = f1Tb, lab1b
                    fbT, labBb = f2T, lab2B
                    na_row, nb_row = nblk[0], nrow[1]
                    featB_d = feat2_d
                else:        # rows of f2 vs all f1
                    aTb, labA = f2Tb, lab2b
                    fbT, labBb = f1T, lab1B
                    na_row, nb_row = nblk[1], nrow[0]
                    featB_d = feat1_d

                # stationary/moving operands of the K=2 norm-correction matmul
                aTn = sb.tile([P, P], f32, name=f"aTn{h}_sb")
                nc.scalar.mul(aTn[:], aTb[:], -2.0)
                stat2 = sb.tile([2, P], f32, name=f"stat2{h}_sb")
                nc.scalar.copy(stat2[0:1, :], na_row[:])
                nc.sync.dma_start(stat2[1:2, :], ones_row[0:1, 0:P])
                mov2 = sb.tile([2, B], f32, name=f"mov2{h}_sb")
                nc.scalar.copy(mov2[0:1, :], ones_row[:])
                nc.sync.dma_start(mov2[1:2, :], nb_row[:])

                # dist^2 = -2*A^T B + na[i] + nb[j], then dis = sqrt(. + eps)
                dis = sb.tile([P, B], f32, name=f"dis{h}_sb")
                for c in range(2):
                    cs = slice(c * 512, (c + 1) * 512)
                    pd2 = ps.tile([P, 512], f32, name=f"pd2_{h}{c}",
                                  tag="pd2", bufs=4)
                    nc.tensor.matmul(pd2[:], lhsT=aTn[:], rhs=fbT[:, cs],
                                     start=True, stop=False)
                    nc.tensor.matmul(pd2[:], lhsT=stat2[:], rhs=mov2[:, cs],
                                     start=False, stop=True)
                    nc.scalar.activation(dis[:, cs], pd2[:],
                                         mybir.ActivationFunctionType.Sqrt,
                                         bias=eps_t[:])

                # masks: s = (labA[i] == labB[j]);  sneg = -1e5 * s
                s_t = sb.tile([P, B], f32, name=f"s{h}_sb")
                nc.gpsimd.tensor_scalar(s_t[:], labBb[:], labA[:], None,
                                        op0=mybir.AluOpType.is_equal)
                sneg = sb.tile([P, B], f32, name=f"sneg{h}_sb")
                nc.gpsimd.tensor_scalar(sneg[:], labBb[:], labA[:],
                                        -NEG_OFFSET,
                                        op0=mybir.AluOpType.is_equal,
                                        op1=mybir.AluOpType.mult)

                pos = sb.tile([P, B], f32, name=f"pos{h}_sb")
                nc.vector.tensor_tensor(out=pos[:], in0=dis[:], in1=s_t[:],
                                        op=mybir.AluOpType.mult)
                negm = sb.tile([P, B], f32, name=f"negm{h}_sb")
                nc.vector.tensor_tensor(out=negm[:], in0=sneg[:], in1=dis[:],
                                        op=mybir.AluOpType.subtract)

                # row stats: furthest positive / closest negative (+indices)
                mx8p = sb.tile([P, 8], f32, name=f"mx8p{h}_sb")
                nc.vector.max(mx8p[:], pos[:])
                idxp = sb.tile([P, 8], u32, name=f"idxp{h}_sb")
                nc.vector.max_index(idxp[:], mx8p[:], pos[:])
                mx8n = sb.tile([P, 8], f32, name=f"mx8n{h}_sb")
                nc.vector.max(mx8n[:], negm[:])
                idxn = sb.tile([P, 8], u32, name=f"idxn{h}_sb")
                nc.vector.max_index(idxn[:], mx8n[:], negm[:])

                # cross term: relu(fp - cn + margin) = relu(mx8p0 + mx8n0 + m)
                cadd = sb.tile([P, 1], f32, name=f"cadd{h}_sb")
                nc.vector.tensor_tensor(out=cadd[:], in0=mx8p[:, 0:1],
                                        in1=mx8n[:, 0:1],
                                        op=mybir.AluOpType.add)
                nc.scalar.activation(stats[:, h:h + 1], cadd[:],
                                     mybir.ActivationFunctionType.Relu,
                                     bias=margin_t[:])

                # intra term: gather the two fB rows per i, distance, hinge
                gP = sb.tile([P, P], f32, name=f"gP{h}_sb")
                nc.gpsimd.indirect_dma_start(
                    out=gP[:], out_offset=None, in_=featB_d[:],
                    in_offset=bass.IndirectOffsetOnAxis(ap=idxp[:, 0:1], axis=0))
                gN = sb.tile([P, P], f32, name=f"gN{h}_sb")
                nc.gpsimd.indirect_dma_start(
                    out=gN[:], out_offset=None, in_=featB_d[:],
                    in_offset=bass.IndirectOffsetOnAxis(ap=idxn[:, 0:1], axis=0))
                diff = sb.tile([P, P], f32, name=f"diff{h}_sb")
                nc.vector.tensor_tensor(out=diff[:], in0=gP[:], in1=gN[:],
                                        op=mybir.AluOpType.subtract)
                dsq = sb.tile([P, P], f32, name=f"dsq{h}_sb")
                nc.scalar.square(dsq[:], diff[:])
                ssq = sb.tile([P, 1], f32, name=f"ssq{h}_sb")
                nc.vector.reduce_sum(ssq[:], dsq[:], axis=mybir.AxisListType.X)
                gd = sb.tile([P, 1], f32, name=f"gd{h}_sb")
                nc.scalar.activation(gd[:], ssq[:],
                                     mybir.ActivationFunctionType.Sqrt,
                                     bias=eps_t[:])
                nc.scalar.activation(stats[:, 2 + h:3 + h], gd[:],
                                     mybir.ActivationFunctionType.Relu,
                                     scale=-1.0, bias=intram_t[:])

            # ---- weighted partial sums + AllGather + final scalar ----
            w_ct = sb.tile([P, 1], f32, name="w_ct_sb")
            nc.vector.memset(w_ct[:], 1.0 / B)
            w_it = sb.tile([P, 1], f32, name="w_it_sb")
            nc.vector.memset(w_it[:], LAMDA / B)
            pfin = ps.tile([1, 4], f32, name="pfin")
            nc.tensor.matmul(pfin[0:1, 0:2], lhsT=w_ct[:], rhs=stats[:, 0:2],
                             start=True, stop=True)
            nc.tensor.matmul(pfin[0:1, 2:4], lhsT=w_it[:], rhs=stats[:, 2:4],
                             start=True, stop=True, skip_group_check=True)

            part = sb.tile([1, 16], f32, name="part_sb")
            nc.vector.memset(part[:], 0.0)
            nc.scalar.copy(part[0:1, 0:4], pfin[:])

            partial_d = dr.tile([1, 16], f32, name="partial_d")
            gath_d = dr.tile([NCORES, 16], f32, name="gath_d",
                             addr_space="Shared")
            nc.gpsimd.dma_start(partial_d[:], part[:])
            nc.gpsimd.collective_compute(
                "AllGather",
                mybir.AluOpType.bypass,
                replica_groups=[list(range(NCORES))],
                ins=[partial_d[:]],
                outs=[gath_d[:]],
            )
            gath = sb.tile([1, NCORES * 16], f32, name="gath_sb")
            nc.gpsimd.dma_start(gath[:],
                                gath_d[:].rearrange("a b -> (a b)")[None, :])
            loss = sb.tile([1, 1], f32, name="loss_sb")
            nc.vector.reduce_sum(loss[:], gath[:], axis=mybir.AxisListType.X)
            nc.sync.dma_start(out_d[:], loss[:])

    nc.compile()
    return nc


def _get_nc():
    if "nc" not in _CACHE:
        _CACHE["nc"] = _build()
    return _CACHE["nc"]


def _in_maps(feat, label1, label2):
    feat = np.asarray(feat, dtype=np.float32)
    f1 = np.ascontiguousarray(feat[:B])
    f2 = np.ascontiguousarray(feat[B:])
    f1T = np.ascontiguousarray(f1.T)
    f2T = np.ascontiguousarray(f2.T)
    l1 = np.asarray(label1).astype(np.float32)
    l2 = np.asarray(label2).astype(np.float32)
    maps = []
    for k in range(NCORES):
        blk = slice(k * P, (k + 1) * P)
        maps.append({
            "f1T": f1T,
            "f2T": f2T,
            "feat1": f1,
            "feat2": f2,
            "lab1r": l1.reshape(1, B),
            "lab2r": l2.reshape(1, B),
            "f1Tb": np.ascontiguousarray(f1T[:, blk]),
            "f2Tb": np.ascontiguousarray(f2T[:, blk]),
            "lab1b": np.ascontiguousarray(l1[blk].reshape(P, 1)),
            "lab2b": np.ascontiguousarray(l2[blk].reshape(P, 1)),
        })
    return maps


def _run(feat, label1, label2, trace=False):
    from concourse.bass_utils import run_bass_kernel_spmd

    nc = _get_nc()
    res = run_bass_kernel_spmd(nc, _in_maps(feat, label1, label2),
                               core_ids=list(range(NCORES)), trace=trace)
    val = np.float32(res.results[0]["out"].reshape(-1)[0])
    return val, res.exec_time_ns


def kernel(feat, label1, label2):
    val, _ = _run(feat, label1, label2)
    return np.array(val, dtype=np.float32)


# revision 8
# speedup vs baseline: 1.5166x; 1.5166x over previous
"""Bass/Trainium2 distributed kernel for nn_BiDirectionalLoss.

Reference computation:
    feature1 = feat[:1024], feature2 = feat[1024:]
    dis = cdist(f1, f2)                                   # [B, B]
    half-1: row-wise masked max/argmax & min/argmin of dis over columns
    half-2: identical with roles swapped -> column-wise stats of the SAME dis
    cross  = mean(relu(furthest_pos - closest_neg + 0.5))
    intra  = mean(relu(0.1 - dis_intra[pos_pos, pos_neg]))
    loss   = cross1 + cross2 + 0.5 * (intra1 + intra2)

Distribution: core k owns row-block k of dis (half-1) and column-block k
(half-2).  Everything is fused into one PSUM matrix per half:

    Q[i,j] = ||f_b[j]||^2 - 2 <a_i, f_b[j]> + BIG * (lab_a[i] == lab_b[j])

built by three accumulating matmuls (all-ones x squares, -2A^T x B, and a
bf16 one-hot x one-hot product for the label mask).  Then, per row of Q:
    max  -> matched entries sit above BIG     -> furthest positive
    min  -> unmatched entries stay below BIG  -> closest negative
    max_index on max/min values -> argmax / argmin (first occurrence)
with the row-constant ||a_i||^2 added back on the [128,1] results.
The intra terms gather two feature rows per index via indirect DMA.
A single 64B-per-core AllGather + local reduce yields the scalar loss.
"""

import sys

if "/opt/trn_rl_repo" not in sys.path:
    sys.path.insert(0, "/opt/trn_rl_repo")

import numpy as np

P = 128          # partitions / block rows per core
B = 1024         # batch size
NCORES = 8
MARGIN = 0.5
NEG_OFFSET = 100000.0
INTRA_MARGIN = 0.1
EPS = 1e-12
LAMDA = 0.5
BIG = 8192.0     # label-mask offset; > any dist^2 here, exact in bf16

_CACHE = {}


def _build():
    import concourse.bacc as bacc
    import concourse.bass as bass
    import concourse.mybir as mybir
    import concourse.tile as tile

    f32 = mybir.dt.float32
    bf16 = mybir.dt.bfloat16
    i32 = mybir.dt.int32
    u32 = mybir.dt.uint32
    AF = mybir.ActivationFunctionType
    ALU = mybir.AluOpType

    nc = bacc.Bacc("TRN2", target_bir_lowering=False, debug=False,
                   num_devices=NCORES)

    # Shared (identical on every core) inputs.
    f1T_d = nc.dram_tensor("f1T", [P, B], f32, kind="ExternalInput")
    f2T_d = nc.dram_tensor("f2T", [P, B], f32, kind="ExternalInput")
    feat1_d = nc.dram_tensor("feat1", [B, P], f32, kind="ExternalInput")
    feat2_d = nc.dram_tensor("feat2", [B, P], f32, kind="ExternalInput")
    lab1r_d = nc.dram_tensor("lab1r", [1, B], f32, kind="ExternalInput")
    lab2r_d = nc.dram_tensor("lab2r", [1, B], f32, kind="ExternalInput")
    # Per-core (sharded) inputs.
    a0m2_d = nc.dram_tensor("a0m2", [P, P], f32, kind="ExternalInput")
    a1m2_d = nc.dram_tensor("a1m2", [P, P], f32, kind="ExternalInput")
    labA0_d = nc.dram_tensor("labA0", [1, P], f32, kind="ExternalInput")
    labA1_d = nc.dram_tensor("labA1", [1, P], f32, kind="ExternalInput")
    fblk1_d = nc.dram_tensor("fblk1", [P, P], f32, kind="ExternalInput")
    fblk2_d = nc.dram_tensor("fblk2", [P, P], f32, kind="ExternalInput")
    out_d = nc.dram_tensor("out", [1, 1], f32, kind="ExternalOutput")

    with tile.TileContext(nc) as tc:
        with (
            tc.tile_pool(name="sb", bufs=1) as sb,
            tc.tile_pool(name="ps", space="PSUM", bufs=1) as ps,
            tc.tile_pool(name="dr", space="DRAM", bufs=1) as dr,
        ):
            # ---- loads ----
            f1T = sb.tile([P, B], f32, name="f1T_sb")
            f2T = sb.tile([P, B], f32, name="f2T_sb")
            a0m2 = sb.tile([P, P], f32, name="a0m2_sb")
            a1m2 = sb.tile([P, P], f32, name="a1m2_sb")
            fblk1 = sb.tile([P, P], f32, name="fblk1_sb")
            fblk2 = sb.tile([P, P], f32, name="fblk2_sb")
            labA0b = sb.tile([P, P], f32, name="labA0b_sb")
            labA1b = sb.tile([P, P], f32, name="labA1b_sb")
            labB0b = sb.tile([P, B], f32, name="labB0b_sb")
            labB1b = sb.tile([P, B], f32, name="labB1b_sb")
            nc.sync.dma_start(f1T[:], f1T_d[:])
            nc.sync.dma_start(f2T[:], f2T_d[:])
            nc.sync.dma_start(a0m2[:], a0m2_d[:])
            nc.sync.dma_start(a1m2[:], a1m2_d[:])
            nc.sync.dma_start(fblk1[:], fblk1_d[:])
            nc.sync.dma_start(fblk2[:], fblk2_d[:])
            # label rows broadcast down all 128 partitions (stride-0 reads)
            nc.sync.dma_start(labA0b[:], labA0_d[0:1, :].to_broadcast([P, P]))
            nc.sync.dma_start(labA1b[:], labA1_d[0:1, :].to_broadcast([P, P]))
            nc.sync.dma_start(labB0b[:], lab2r_d[0:1, :].to_broadcast([P, B]))
            nc.sync.dma_start(labB1b[:], lab1r_d[0:1, :].to_broadcast([P, B]))

            ones = sb.tile([P, P], f32, name="ones_sb")
            nc.vector.memset(ones[:], 1.0)
            eps_t = sb.tile([P, 1], f32, name="eps_sb")
            nc.vector.memset(eps_t[:], EPS)
            margin_t = sb.tile([P, 1], f32, name="margin_sb")
            nc.vector.memset(margin_t[:], MARGIN)
            intram_t = sb.tile([P, 1], f32, name="intram_sb")
            nc.vector.memset(intram_t[:], INTRA_MARGIN)

            # partition-index column (value = partition id), as f32
            iotap_i = sb.tile([P, 1], i32, name="iotap_i_sb")
            nc.gpsimd.iota(iotap_i[:], pattern=[[0, 1]], base=0,
                           channel_multiplier=1)
            iotap = sb.tile([P, 1], f32, name="iotap_sb")
            nc.vector.tensor_copy(iotap[:], iotap_i[:])

            # one-hot label matrices (bf16): ohA[c,i] = (labA[i]==c),
            # ohB[c,j] = BIG * (labB[j]==c)
            ohA = []
            ohB = []
            for h, (lAb, lBb) in enumerate(((labA0b, labB0b),
                                            (labA1b, labB1b))):
                oa = sb.tile([P, P], bf16, name=f"ohA{h}_sb")
                nc.vector.tensor_scalar(oa[:], lAb[:], iotap[:], None,
                                        op0=ALU.is_equal)
                ob = sb.tile([P, B], bf16, name=f"ohB{h}_sb")
                nc.vector.tensor_scalar(ob[:], lBb[:], iotap[:], BIG,
                                        op0=ALU.is_equal, op1=ALU.mult)
                ohA.append(oa)
                ohB.append(ob)

            # squared features (rhs of the all-ones norm matmul)
            sq1 = sb.tile([P, B], f32, name="sq1_sb")
            nc.scalar.square(sq1[:], f1T[:])
            sq2 = sb.tile([P, B], f32, name="sq2_sb")
            nc.scalar.square(sq2[:], f2T[:])

            # per-row block norms ||a_i||^2 -> [128, 1]
            na2 = []
            for s_i, fb in ((0, fblk1), (1, fblk2)):
                sqb = sb.tile([P, P], f32, name=f"sqb{s_i}_sb")
                nc.scalar.square(sqb[:], fb[:])
                nn = sb.tile([P, 1], f32, name=f"na2_{s_i}_sb")
                nc.vector.reduce_sum(nn[:], sqb[:], axis=mybir.AxisListType.X)
                na2.append(nn)

            # ---- per-half pipeline ----
            stats = sb.tile([P, 4], f32, name="stats_sb")  # [ct0, ct1, it0, it1]
            for h in range(2):
                if h == 0:   # rows of f1 vs all f2
                    am2, sqb_full, na_col = a0m2, sq2, na2[0]
                    fbT, featB_d = f2T, feat2_d
                else:        # rows of f2 vs all f1
                    am2, sqb_full, na_col = a1m2, sq1, na2[1]
                    fbT, featB_d = f1T, feat1_d

                # Q = nb2[j] - 2<a_i, b_j> + BIG*s  (accumulated in PSUM)
                Q = ps.tile([P, B], f32, name=f"Q{h}", tag="Q", bufs=2)
                for c in range(2):
                    cs = slice(c * 512, (c + 1) * 512)
                    nc.tensor.matmul(Q[:, cs], lhsT=ones[:],
                                     rhs=sqb_full[:, cs],
                                     start=True, stop=False)
                    nc.tensor.matmul(Q[:, cs], lhsT=am2[:], rhs=fbT[:, cs],
                                     start=False, stop=False)
                    nc.tensor.matmul(Q[:, cs], lhsT=ohA[h][:],
                                     rhs=ohB[h][:, cs],
                                     start=False, stop=True)

                # row stats straight off PSUM
                maxv = sb.tile([P, 1], f32, name=f"maxv{h}_sb")
                nc.vector.tensor_reduce(maxv[:], Q[:], op=ALU.max,
                                        axis=mybir.AxisListType.X)
                minv = sb.tile([P, 1], f32, name=f"minv{h}_sb")
                nc.vector.tensor_reduce(minv[:], Q[:], op=ALU.min,
                                        axis=mybir.AxisListType.X)
                idxp = sb.tile([P, 8], u32, name=f"idxp{h}_sb")
                nc.vector.max_index(idxp[:], maxv[:].to_broadcast([P, 8]), Q[:])
                idxn = sb.tile([P, 8], u32, name=f"idxn{h}_sb")
                nc.vector.max_index(idxn[:], minv[:].to_broadcast([P, 8]), Q[:])

                # fp = sqrt(relu(maxv + na2 - BIG) + eps)
                # cn = sqrt(relu(minv + na2) + eps)
                nb_bias = sb.tile([P, 1], f32, name=f"nb_bias{h}_sb")
                nc.vector.tensor_scalar(nb_bias[:], na_col[:], -BIG, None,
                                        op0=ALU.add)
                fp2 = sb.tile([P, 1], f32, name=f"fp2_{h}_sb")
                nc.scalar.activation(fp2[:], maxv[:], AF.Relu, bias=nb_bias[:])
                fp = sb.tile([P, 1], f32, name=f"fp{h}_sb")
                nc.scalar.activation(fp[:], fp2[:], AF.Sqrt, bias=eps_t[:])
                cn2 = sb.tile([P, 1], f32, name=f"cn2_{h}_sb")
                nc.scalar.activation(cn2[:], minv[:], AF.Relu, bias=na_col[:])
                cn = sb.tile([P, 1], f32, name=f"cn{h}_sb")
                nc.scalar.activation(cn[:], cn2[:], AF.Sqrt, bias=eps_t[:])

                # cross term: relu(fp - cn + margin)
                cd = sb.tile([P, 1], f32, name=f"cd{h}_sb")
                nc.vector.tensor_tensor(out=cd[:], in0=fp[:], in1=cn[:],
                                        op=ALU.subtract)
                nc.scalar.activation(stats[:, h:h + 1], cd[:], AF.Relu,
                                     bias=margin_t[:])

                # intra term: gather the two fB rows per i, distance, hinge
                gP = sb.tile([P, P], f32, name=f"gP{h}_sb")
                nc.gpsimd.indirect_dma_start(
                    out=gP[:], out_offset=None, in_=featB_d[:],
                    in_offset=bass.IndirectOffsetOnAxis(ap=idxp[:, 0:1], axis=0))
                gN = sb.tile([P, P], f32, name=f"gN{h}_sb")
                nc.gpsimd.indirect_dma_start(
                    out=gN[:], out_offset=None, in_=featB_d[:],
                    in_offset=bass.IndirectOffsetOnAxis(ap=idxn[:, 0:1], axis=0))
                diff = sb.tile([P, P], f32, name=f"diff{h}_sb")
                nc.vector.tensor_tensor(out=diff[:], in0=gP[:], in1=gN[:],
                                        op=ALU.subtract)
                dsq = sb.tile([P, P], f32, name=f"dsq{h}_sb")
                nc.scalar.square(dsq[:], diff[:])
                ssq = sb.tile([P, 1], f32, name=f"ssq{h}_sb")
                nc.vector.reduce_sum(ssq[:], dsq[:], axis=mybir.AxisListType.X)
                gd = sb.tile([P, 1], f32, name=f"gd{h}_sb")
                nc.scalar.activation(gd[:], ssq[:], AF.Sqrt, bias=eps_t[:])
                nc.scalar.activation(stats[:, 2 + h:3 + h], gd[:], AF.Relu,
                                     scale=-1.0, bias=intram_t[:])

            # ---- weighted partial sums + AllGather + final scalar ----
            w_ct = sb.tile([P, 1], f32, name="w_ct_sb")
            nc.vector.memset(w_ct[:], 1.0 / B)
            w_it = sb.tile([P, 1], f32, name="w_it_sb")
            nc.vector.memset(w_it[:], LAMDA / B)
            pfin = ps.tile([1, 4], f32, name="pfin")
            nc.tensor.matmul(pfin[0:1, 0:2], lhsT=w_ct[:], rhs=stats[:, 0:2],
                             start=True, stop=True)
            nc.tensor.matmul(pfin[0:1, 2:4], lhsT=w_it[:], rhs=stats[:, 2:4],
                             start=True, stop=True, skip_group_check=True)

            part = sb.tile([1, 16], f32, name="part_sb")
            nc.vector.memset(part[:], 0.0)
            nc.scalar.copy(part[0:1, 0:4], pfin[:])

            partial_d = dr.tile([1, 16], f32, name="partial_d")
            gath_d = dr.tile([NCORES, 16], f32, name="gath_d",
                             addr_space="Shared")
            nc.gpsimd.dma_start(partial_d[:], part[:])
            nc.gpsimd.collective_compute(
                "AllGather",
                mybir.AluOpType.bypass,
                replica_groups=[list(range(NCORES))],
                ins=[partial_d[:]],
                outs=[gath_d[:]],
            )
            gath = sb.tile([1, NCORES * 16], f32, name="gath_sb")
            nc.gpsimd.dma_start(gath[:],
                                gath_d[:].rearrange("a b -> (a b)")[None, :])
            loss = sb.tile([1, 1], f32, name="loss_sb")
            nc.vector.reduce_sum(loss[:], gath[:], axis=mybir.AxisListType.X)
            nc.sync.dma_start(out_d[:], loss[:])

    nc.compile()
    return nc


def _get_nc():
    if "nc" not in _CACHE:
        _CACHE["nc"] = _build()
    return _CACHE["nc"]


def _in_maps(feat, label1, label2):
    feat = np.asarray(feat, dtype=np.float32)
    f1 = np.ascontiguousarray(feat[:B])
    f2 = np.ascontiguousarray(feat[B:])
    f1T = np.ascontiguousarray(f1.T)
    f2T = np.ascontiguousarray(f2.T)
    l1 = np.asarray(label1).astype(np.float32)
    l2 = np.asarray(label2).astype(np.float32)
    maps = []
    for k in range(NCORES):
        blk = slice(k * P, (k + 1) * P)
        maps.append({
            "f1T": f1T,
            "f2T": f2T,
            "feat1": f1,
            "feat2": f2,
            "lab1r": l1.reshape(1, B),
            "lab2r": l2.reshape(1, B),
            "a0m2": np.ascontiguousarray(-2.0 * f1T[:, blk]),
            "a1m2": np.ascontiguousarray(-2.0 * f2T[:, blk]),
            "labA0": np.ascontiguousarray(l1[blk].reshape(1, P)),
            "labA1": np.ascontiguousarray(l2[blk].reshape(1, P)),
            "fblk1": np.ascontiguousarray(f1[blk]),
            "fblk2": np.ascontiguousarray(f2[blk]),
        })
    return maps


def _run(feat, label1, label2, trace=False):
    from concourse.bass_utils import run_bass_kernel_spmd

    nc = _get_nc()
    res = run_bass_kernel_spmd(nc, _in_maps(feat, label1, label2),
                               core_ids=list(range(NCORES)), trace=trace)
    val = np.float32(res.results[0]["out"].reshape(-1)[0])
    return val, res.exec_time_ns


def kernel(feat, label1, label2):
    val, _ = _run(feat, label1, label2)
    return np.array(val, dtype=np.float32)


# revision 11
# speedup vs baseline: 1.9115x; 1.2603x over previous
"""Bass/Trainium2 distributed kernel for nn_BiDirectionalLoss.

Reference computation:
    feature1 = feat[:1024], feature2 = feat[1024:]
    dis = cdist(f1, f2)                                   # [B, B]
    half-1: row-wise masked max/argmax & min/argmin of dis over columns
    half-2: identical with roles swapped -> column-wise stats of the SAME dis
    cross  = mean(relu(furthest_pos - closest_neg + 0.5))
    intra  = mean(relu(0.1 - dis_intra[pos_pos, pos_neg]))
    loss   = cross1 + cross2 + 0.5 * (intra1 + intra2)

Distribution: core k owns row-block k of dis (half-1) and column-block k
(half-2).  Everything is fused into one PSUM matrix per half:

    Q[i,j] = ||f_b[j]||^2 - 2 <a_i, f_b[j]> + BIG * (lab_a[i] == lab_b[j])

built by three accumulating matmuls (all-ones x squares, -2A^T x B, and a
bf16 one-hot x one-hot product for the label mask).  Then, per row of Q:
    max  -> matched entries sit above BIG     -> furthest positive
    min  -> unmatched entries stay below BIG  -> closest negative
    max_index on max/min values -> argmax / argmin (first occurrence)
with the row-constant ||a_i||^2 added back on the [128,1] results.
The intra terms gather two feature rows per index via indirect DMA.
A single 64B-per-core AllGather + local reduce yields the scalar loss.
"""

import sys

if "/opt/trn_rl_repo" not in sys.path:
    sys.path.insert(0, "/opt/trn_rl_repo")

import numpy as np

P = 128          # partitions / block rows per core
B = 1024         # batch size
NCORES = 8
MARGIN = 0.5
NEG_OFFSET = 100000.0
INTRA_MARGIN = 0.1
EPS = 1e-12
LAMDA = 0.5
BIG = 8192.0     # label-mask offset; > any dist^2 here, exact in bf16

_CACHE = {}


def _build():
    import concourse.bacc as bacc
    import concourse.bass as bass
    import concourse.mybir as mybir
    import concourse.tile as tile

    f32 = mybir.dt.float32
    bf16 = mybir.dt.bfloat16
    i32 = mybir.dt.int32
    u32 = mybir.dt.uint32
    AF = mybir.ActivationFunctionType
    ALU = mybir.AluOpType

    nc = bacc.Bacc("TRN2", target_bir_lowering=False, debug=False,
                   num_devices=NCORES)

    # Shared (identical on every core) inputs.
    f1T_d = nc.dram_tensor("f1T", [P, B], bf16, kind="ExternalInput")
    f2T_d = nc.dram_tensor("f2T", [P, B], bf16, kind="ExternalInput")
    feat1_d = nc.dram_tensor("feat1", [B, P], f32, kind="ExternalInput")
    feat2_d = nc.dram_tensor("feat2", [B, P], f32, kind="ExternalInput")
    lab1r_d = nc.dram_tensor("lab1r", [1, B], bf16, kind="ExternalInput")
    lab2r_d = nc.dram_tensor("lab2r", [1, B], bf16, kind="ExternalInput")
    # Per-core (sharded) inputs.
    a0m2_d = nc.dram_tensor("a0m2", [P, P], bf16, kind="ExternalInput")
    a1m2_d = nc.dram_tensor("a1m2", [P, P], bf16, kind="ExternalInput")
    labA0_d = nc.dram_tensor("labA0", [1, P], bf16, kind="ExternalInput")
    labA1_d = nc.dram_tensor("labA1", [1, P], bf16, kind="ExternalInput")
    fblk1_d = nc.dram_tensor("fblk1", [P, P], f32, kind="ExternalInput")
    fblk2_d = nc.dram_tensor("fblk2", [P, P], f32, kind="ExternalInput")
    out_d = nc.dram_tensor("out", [1, 1], f32, kind="ExternalOutput")

    with tile.TileContext(nc) as tc:
        with (
            tc.tile_pool(name="sb", bufs=1) as sb,
            tc.tile_pool(name="ps", space="PSUM", bufs=1) as ps,
            tc.tile_pool(name="dr", space="DRAM", bufs=1) as dr,
        ):
            # ---- loads ----
            f1T = sb.tile([P, B], bf16, name="f1T_sb")
            f2T = sb.tile([P, B], bf16, name="f2T_sb")
            a0m2 = sb.tile([P, P], bf16, name="a0m2_sb")
            a1m2 = sb.tile([P, P], bf16, name="a1m2_sb")
            fblk1 = sb.tile([P, P], f32, name="fblk1_sb")
            fblk2 = sb.tile([P, P], f32, name="fblk2_sb")
            labA0b = sb.tile([P, P], bf16, name="labA0b_sb")
            labA1b = sb.tile([P, P], bf16, name="labA1b_sb")
            labB0b = sb.tile([P, B], bf16, name="labB0b_sb")
            labB1b = sb.tile([P, B], bf16, name="labB1b_sb")
            # spread loads across engine DMA queues; label rows first (they
            # gate the one-hot builds feeding the mask matmul)
            nc.scalar.dma_start(labB0b[:], lab2r_d[0:1, :].to_broadcast([P, B]))
            nc.sync.dma_start(labB1b[:], lab1r_d[0:1, :].to_broadcast([P, B]))
            nc.gpsimd.dma_start(labA0b[:], labA0_d[0:1, :].to_broadcast([P, P]))
            nc.gpsimd.dma_start(labA1b[:], labA1_d[0:1, :].to_broadcast([P, P]))
            nc.sync.dma_start(f1T[:], f1T_d[:])
            nc.gpsimd.dma_start(f2T[:], f2T_d[:])
            nc.scalar.dma_start(a0m2[:], a0m2_d[:])
            nc.gpsimd.dma_start(a1m2[:], a1m2_d[:])
            nc.scalar.dma_start(fblk1[:], fblk1_d[:])
            nc.sync.dma_start(fblk2[:], fblk2_d[:])

            ones = sb.tile([P, P], bf16, name="ones_sb")
            nc.vector.memset(ones[:], 1.0)
            eps_t = sb.tile([P, 1], f32, name="eps_sb")
            nc.vector.memset(eps_t[:], EPS)
            margin_t = sb.tile([P, 1], f32, name="margin_sb")
            nc.vector.memset(margin_t[:], MARGIN)
            intram_t = sb.tile([P, 1], f32, name="intram_sb")
            nc.vector.memset(intram_t[:], INTRA_MARGIN)

            # partition-index column (value = partition id), as f32
            iotap_i = sb.tile([P, 1], i32, name="iotap_i_sb")
            nc.gpsimd.iota(iotap_i[:], pattern=[[0, 1]], base=0,
                           channel_multiplier=1)
            iotap = sb.tile([P, 1], f32, name="iotap_sb")
            nc.vector.tensor_copy(iotap[:], iotap_i[:])

            # one-hot label matrices (bf16): ohA[c,i] = (labA[i]==c),
            # ohB[c,j] = BIG * (labB[j]==c)
            ohA = []
            ohB = []
            for h, (lAb, lBb) in enumerate(((labA0b, labB0b),
                                            (labA1b, labB1b))):
                oa = sb.tile([P, P], bf16, name=f"ohA{h}_sb")
                nc.vector.tensor_scalar(oa[:], lAb[:], iotap[:], None,
                                        op0=ALU.is_equal)
                ob = sb.tile([P, B], bf16, name=f"ohB{h}_sb")
                nc.vector.tensor_scalar(ob[:], lBb[:], iotap[:], BIG,
                                        op0=ALU.is_equal, op1=ALU.mult)
                ohA.append(oa)
                ohB.append(ob)

            # squared features (rhs of the all-ones norm matmul)
            sq1 = sb.tile([P, B], bf16, name="sq1_sb")
            nc.scalar.square(sq1[:], f1T[:])
            sq2 = sb.tile([P, B], bf16, name="sq2_sb")
            nc.scalar.square(sq2[:], f2T[:])

            # per-row block norms ||a_i||^2 -> [128, 1]
            na2 = []
            for s_i, fb in ((0, fblk1), (1, fblk2)):
                sqb = sb.tile([P, P], f32, name=f"sqb{s_i}_sb")
                nc.scalar.square(sqb[:], fb[:])
                nn = sb.tile([P, 1], f32, name=f"na2_{s_i}_sb")
                nc.vector.reduce_sum(nn[:], sqb[:], axis=mybir.AxisListType.X)
                na2.append(nn)

            # ---- per-half pipeline ----
            stats = sb.tile([P, 4], f32, name="stats_sb")  # [ct0, ct1, it0, it1]
            for h in range(2):
                if h == 0:   # rows of f1 vs all f2
                    am2, sqb_full, na_col = a0m2, sq2, na2[0]
                    fbT, featB_d = f2T, feat2_d
                else:        # rows of f2 vs all f1
                    am2, sqb_full, na_col = a1m2, sq1, na2[1]
                    fbT, featB_d = f1T, feat1_d

                # Q = nb2[j] - 2<a_i, b_j> + BIG*s  (accumulated in PSUM)
                Q = ps.tile([P, B], f32, name=f"Q{h}", tag="Q", bufs=2)
                for c in range(2):
                    cs = slice(c * 512, (c + 1) * 512)
                    nc.tensor.matmul(Q[:, cs], lhsT=ones[:],
                                     rhs=sqb_full[:, cs],
                                     start=True, stop=False)
                    nc.tensor.matmul(Q[:, cs], lhsT=am2[:], rhs=fbT[:, cs],
                                     start=False, stop=False)
                    nc.tensor.matmul(Q[:, cs], lhsT=ohA[h][:],
                                     rhs=ohB[h][:, cs],
                                     start=False, stop=True)

                # row stats straight off PSUM
                maxv = sb.tile([P, 1], f32, name=f"maxv{h}_sb")
                nc.vector.tensor_reduce(maxv[:], Q[:], op=ALU.max,
                                        axis=mybir.AxisListType.X)
                minv = sb.tile([P, 1], f32, name=f"minv{h}_sb")
                nc.vector.tensor_reduce(minv[:], Q[:], op=ALU.min,
                                        axis=mybir.AxisListType.X)
                idxp = sb.tile([P, 8], u32, name=f"idxp{h}_sb")
                nc.vector.max_index(idxp[:], maxv[:].to_broadcast([P, 8]), Q[:])
                idxn = sb.tile([P, 8], u32, name=f"idxn{h}_sb")
                nc.vector.max_index(idxn[:], minv[:].to_broadcast([P, 8]), Q[:])

                # fp = sqrt(relu(maxv + na2 - BIG) + eps)
                # cn = sqrt(relu(minv + na2) + eps)
                nb_bias = sb.tile([P, 1], f32, name=f"nb_bias{h}_sb")
                nc.vector.tensor_scalar(nb_bias[:], na_col[:], -BIG, None,
                                        op0=ALU.add)
                fp2 = sb.tile([P, 1], f32, name=f"fp2_{h}_sb")
                nc.scalar.activation(fp2[:], maxv[:], AF.Relu, bias=nb_bias[:])
                fp = sb.tile([P, 1], f32, name=f"fp{h}_sb")
                nc.scalar.activation(fp[:], fp2[:], AF.Sqrt, bias=eps_t[:])
                cn2 = sb.tile([P, 1], f32, name=f"cn2_{h}_sb")
                nc.scalar.activation(cn2[:], minv[:], AF.Relu, bias=na_col[:])
                cn = sb.tile([P, 1], f32, name=f"cn{h}_sb")
                nc.scalar.activation(cn[:], cn2[:], AF.Sqrt, bias=eps_t[:])

                # cross term: relu(fp - cn + margin)
                cd = sb.tile([P, 1], f32, name=f"cd{h}_sb")
                nc.vector.tensor_tensor(out=cd[:], in0=fp[:], in1=cn[:],
                                        op=ALU.subtract)
                nc.scalar.activation(stats[:, h:h + 1], cd[:], AF.Relu,
                                     bias=margin_t[:])

                # intra term: gather the two fB rows per i, distance, hinge
                gP = sb.tile([P, P], f32, name=f"gP{h}_sb")
                nc.gpsimd.indirect_dma_start(
                    out=gP[:], out_offset=None, in_=featB_d[:],
                    in_offset=bass.IndirectOffsetOnAxis(ap=idxp[:, 0:1], axis=0))
                gN = sb.tile([P, P], f32, name=f"gN{h}_sb")
                nc.gpsimd.indirect_dma_start(
                    out=gN[:], out_offset=None, in_=featB_d[:],
                    in_offset=bass.IndirectOffsetOnAxis(ap=idxn[:, 0:1], axis=0))
                diff = sb.tile([P, P], f32, name=f"diff{h}_sb")
                nc.vector.tensor_tensor(out=diff[:], in0=gP[:], in1=gN[:],
                                        op=ALU.subtract)
                dsq = sb.tile([P, P], f32, name=f"dsq{h}_sb")
                nc.scalar.square(dsq[:], diff[:])
                ssq = sb.tile([P, 1], f32, name=f"ssq{h}_sb")
                nc.vector.reduce_sum(ssq[:], dsq[:], axis=mybir.AxisListType.X)
                gd = sb.tile([P, 1], f32, name=f"gd{h}_sb")
                nc.scalar.activation(gd[:], ssq[:], AF.Sqrt, bias=eps_t[:])
                nc.scalar.activation(stats[:, 2 + h:3 + h], gd[:], AF.Relu,
                                     scale=-1.0, bias=intram_t[:])

            # ---- weighted partial sums + AllGather + final scalar ----
            w_ct = sb.tile([P, 1], f32, name="w_ct_sb")
            nc.vector.memset(w_ct[:], 1.0 / B)
            w_it = sb.tile([P, 1], f32, name="w_it_sb")
            nc.vector.memset(w_it[:], LAMDA / B)
            pfin = ps.tile([1, 4], f32, name="pfin")
            nc.tensor.matmul(pfin[0:1, 0:2], lhsT=w_ct[:], rhs=stats[:, 0:2],
                             start=True, stop=True)
            nc.tensor.matmul(pfin[0:1, 2:4], lhsT=w_it[:], rhs=stats[:, 2:4],
                             start=True, stop=True, skip_group_check=True)

            part = sb.tile([1, 16], f32, name="part_sb")
            nc.vector.memset(part[:], 0.0)
            nc.scalar.copy(part[0:1, 0:4], pfin[:])

            partial_d = dr.tile([1, 16], f32, name="partial_d")
            gath_d = dr.tile([NCORES, 16], f32, name="gath_d",
                             addr_space="Shared")
            nc.gpsimd.dma_start(partial_d[:], part[:])
            nc.gpsimd.collective_compute(
                "AllGather",
                mybir.AluOpType.bypass,
                replica_groups=[list(range(NCORES))],
                ins=[partial_d[:]],
                outs=[gath_d[:]],
            )
            gath = sb.tile([1, NCORES * 16], f32, name="gath_sb")
            nc.gpsimd.dma_start(gath[:],
                                gath_d[:].rearrange("a b -> (a b)")[None, :])
            loss = sb.tile([1, 1], f32, name="loss_sb")
            nc.vector.reduce_sum(loss[:], gath[:], axis=mybir.AxisListType.X)
            nc.sync.dma_start(out_d[:], loss[:])

    nc.compile()
    return nc


def _get_nc():
    if "nc" not in _CACHE:
        _CACHE["nc"] = _build()
    return _CACHE["nc"]


def _in_maps(feat, label1, label2):
    import ml_dtypes
    bf = ml_dtypes.bfloat16
    feat = np.asarray(feat, dtype=np.float32)
    f1 = np.ascontiguousarray(feat[:B])
    f2 = np.ascontiguousarray(feat[B:])
    f1T = np.ascontiguousarray(f1.T)
    f2T = np.ascontiguousarray(f2.T)
    f1Tb = f1T.astype(bf)
    f2Tb = f2T.astype(bf)
    l1 = np.asarray(label1).astype(np.float32)
    l2 = np.asarray(label2).astype(np.float32)
    maps = []
    for k in range(NCORES):
        blk = slice(k * P, (k + 1) * P)
        maps.append({
            "f1T": f1Tb,
            "f2T": f2Tb,
            "feat1": f1,
            "feat2": f2,
            "lab1r": l1.reshape(1, B).astype(bf),
            "lab2r": l2.reshape(1, B).astype(bf),
            "a0m2": np.ascontiguousarray(-2.0 * f1T[:, blk]).astype(bf),
            "a1m2": np.ascontiguousarray(-2.0 * f2T[:, blk]).astype(bf),
            "labA0": np.ascontiguousarray(l1[blk].reshape(1, P)).astype(bf),
            "labA1": np.ascontiguousarray(l2[blk].reshape(1, P)).astype(bf),
            "fblk1": np.ascontiguousarray(f1[blk]),
            "fblk2": np.ascontiguousarray(f2[blk]),
        })
    return maps


def _run(feat, label1, label2, trace=False):
    from concourse.bass_utils import run_bass_kernel_spmd

    nc = _get_nc()
    res = run_bass_kernel_spmd(nc, _in_maps(feat, label1, label2),
                               core_ids=list(range(NCORES)), trace=trace)
    val = np.float32(res.results[0]["out"].reshape(-1)[0])
    return val, res.exec_time_ns


def kernel(feat, label1, label2):
    val, _ = _run(feat, label1, label2)
    return np.array(val, dtype=np.float32)
